# revision 19
# baseline (speedup 1.0000x reference)
"""Trainium2 Bass kernel for Conformer-style relative-position MHSA.

Sharding: data-parallel over batch — B=8 batch elements, one per NeuronCore.
Per core: LN -> QKVP projections -> rel-pos scores -> softmax -> AV -> output
projection -> residual. No collectives.

v2: scores are computed directly TRANSPOSED ([m on partitions, n free]) so
the AV contraction needs no PE transposes (baseline spent ~140us on 512
transpose instructions and HAM-oscillated around them):
  - AC^T[m,n] = k[.,m]^T q'[.,n] — same matmuls as AC with lhsT/rhs swapped.
  - BD needs the Transformer-XL shift, done via an fp8 DRAM round trip
    ([T, T+1] rows with a leading zero column). The shifted+transposed
    read-back uses the DMA X-bar transpose on a bf16 BITCAST of the fp8
    buffer: each 16-bit unit carries the fp8 pair (m=2j, m=2j+1), so the
    transposed SBUF tile holds m interleaved at byte granularity
    (m = 256*c + 2*j + b for xbar chunk c, partition j, byte b).
  - AC matmuls and the V projection select the matching stride-2 token
    subsets so everything downstream lives in the packed (c,j,b) layout.
  - K=64 score matmuls are plain fp8 (no DoubleRow — it only costs extra
    LDWEIGHTS here); the two heads of a pair sit at partition bases 0/64,
    so their matmuls row-group-tile and run concurrently on the PE.
Softmax denominator via an extra ones-column in the AV weights (row O of
the AV psum), divided out after. exp applies the 1/sqrt(64) scale.
fp8 fast path elsewhere as baseline: weights hostside at 16x, projections
DoubleRow over the d=512 contraction, residual path at 256x.
"""

import sys

for _p in ("/opt/trn_rl_repo", "/root/.axon_site/_ro/pypackages"):
    if _p not in sys.path:
        sys.path.insert(0, _p)

import numpy as np
import ml_dtypes

import concourse.bass as bass
import concourse.mybir as mybir
import concourse.tile as tile
from concourse import bacc
from concourse.bass_utils import run_bass_kernel_spmd
from concourse.masks import make_identity

F32 = mybir.dt.float32
BF16 = mybir.dt.bfloat16
FP8 = mybir.dt.float8e4
AX = mybir.AluOpType
AF = mybir.ActivationFunctionType
DR = mybir.MatmulPerfMode.DoubleRow

P = 128
T = 1024
D = 512
H = 8
O = 64
KT = D // P      # 4 k-tiles over model dim
NT = T // P      # 8 tiles over sequence
NCH = T // 512   # 2 free-dim chunks of 512
NC_XB = 4        # xbar chunks of 256 m-values (128 fp8 pairs)
AVP = 80         # avw per-head pitch (ones col at 64; stride % 16 == 0)
NPAIR = H // 2
LN_EPS = 1e-3
SW = 16.0        # weight fp8 scale
SR = 256.0       # residual-path scale (SW*SW)


def build_nc(use_beta=True):
    nc = bacc.Bacc("TRN2", target_bir_lowering=False)

    x_res = nc.dram_tensor("x_res", [P, NT, D], BF16, kind="ExternalInput")
    post = nc.dram_tensor("post", [P, KT, T], FP8, kind="ExternalInput")
    wq = nc.dram_tensor("wq", [P, KT, D], FP8, kind="ExternalInput")
    wk = nc.dram_tensor("wk", [P, KT, D], FP8, kind="ExternalInput")
    wv = nc.dram_tensor("wv", [P, KT, D], FP8, kind="ExternalInput")
    wp = nc.dram_tensor("wp", [P, KT, D], FP8, kind="ExternalInput")
    wo = nc.dram_tensor("wo", [P, KT, D], FP8, kind="ExternalInput")
    u_in = nc.dram_tensor("u_in", [P, KT], F32, kind="ExternalInput")
    v_in = nc.dram_tensor("v_in", [P, KT], F32, kind="ExternalInput")
    if use_beta:
        beta_in = nc.dram_tensor("beta_in", [P, D], BF16,
                                 kind="ExternalInput")
    out = nc.dram_tensor("out", [T, D], BF16, kind="ExternalOutput")

    with tile.TileContext(nc) as tc:
        with (
            tc.tile_pool(name="consts", bufs=1) as consts,
            tc.tile_pool(name="acts", bufs=1) as acts,
            tc.tile_pool(name="dram", bufs=2, space="DRAM") as dram_pool,
        ):
            xres_sb = acts.tile([P, NT, D], BF16)
            nc.sync.dma_start(xres_sb[:], x_res[:])
            if use_beta:
                beta_sb = consts.tile([P, D], BF16, tag="beta")
                nc.sync.dma_start(beta_sb[:], beta_in[:])
            ones_bc = consts.tile([P, O], BF16, tag="ones_bc")
            nc.vector.memset(ones_bc[:], 1.0 / SW)
            eps_sb = consts.tile([P, 1], F32, tag="eps")
            nc.vector.memset(eps_sb[:], LN_EPS * SR * SR)
            ident = consts.tile([P, P], BF16)
            make_identity(nc, ident)
            ident8 = consts.tile([P, P], FP8, tag="ident8")
            make_identity(nc, ident8)

            # projections land transposed: [feat-part, chunk, token]
            qu = acts.tile([P, KT, T], FP8)
            qv = acts.tile([P, KT, T], FP8)
            kT_sb = acts.tile([P, KT, T], FP8)
            pT_sb = acts.tile([P, KT, T], FP8)
            outT = acts.tile([P, KT, T], FP8)
            # AV weights, packed: [j, c, b, h, o] with m = 256c + 2j + b
            avw = acts.tile([P, NC_XB, 2, H, AVP], FP8)
            nc.vector.memset(avw[:], 1.0)

            with (
                tc.tile_pool(name="early", bufs=1) as early,
                tc.tile_pool(name="psP", bufs=2, space="PSUM") as psP,
                tc.tile_pool(name="psV", bufs=2, space="PSUM") as psV,
                tc.tile_pool(name="psB", bufs=2, space="PSUM") as psB,
            ):
                xlnT = early.tile([P, KT, T], FP8)
                xln_nd = early.tile([P, NT, D], BF16)
                with tc.tile_pool(name="ln_tmp", bufs=4) as ln_tmp:
                    with nc.named_scope("ln"):
                        for nt in range(NT):
                            st6 = ln_tmp.tile([P, 6], F32, tag="st6")
                            nc.vector.bn_stats(out=st6[:], in_=xres_sb[:, nt, :])
                            mv = ln_tmp.tile([P, 2], F32, tag="mv")
                            nc.vector.bn_aggr(out=mv[:], in_=st6[:])
                            sd = ln_tmp.tile([P, 1], F32, tag="sd")
                            nc.scalar.activation(out=sd[:], in_=mv[:, 1:2],
                                                 func=AF.Sqrt, bias=eps_sb[:])
                            rstd = ln_tmp.tile([P, 1], F32, tag="rstd")
                            nc.vector.reciprocal(rstd[:], sd[:])
                            nc.gpsimd.tensor_scalar(
                                out=xln_nd[:, nt, :], in0=xres_sb[:, nt, :],
                                scalar1=mv[:, 0:1], scalar2=rstd[:],
                                op0=AX.subtract, op1=AX.mult)
                            if use_beta:
                                nc.gpsimd.tensor_add(
                                    xln_nd[:, nt, :], xln_nd[:, nt, :],
                                    beta_sb[:])
                        for kt in range(KT):
                            ps_x = psB.tile([P, T], BF16, tag="tx")
                            for nt in range(NT):
                                nc.tensor.transpose(
                                    ps_x[:, bass.ts(nt, P)],
                                    xln_nd[:, nt, bass.ts(kt, P)],
                                    ident[:])
                            nc.scalar.copy(xlnT[:, kt, :], ps_x[:])

                post_sb = early.tile([P, KT, T], FP8)
                nc.sync.dma_start(post_sb[:], post[:])
                w_sb = {}
                for name, t in (("wq", wq), ("wk", wk), ("wv", wv), ("wp", wp),
                                ("wo", wo)):
                    w_sb[name] = consts.tile([P, KT, D], FP8, tag=f"w_{name}",
                                             name=f"w_{name}")
                    nc.sync.dma_start(w_sb[name][:], t[:])
                u_sb = consts.tile([P, KT], F32, tag="u")
                nc.sync.dma_start(u_sb[:], u_in[:])
                v_sb = consts.tile([P, KT], F32, tag="v")
                nc.sync.dma_start(v_sb[:], v_in[:])

                # ---- projections (DoubleRow over kt pairs) ----
                def proj_mm(ps, wname, rhs_tile, mch):
                    for nch in range(NCH):
                        for p2 in range(2):
                            nc.tensor.matmul(
                                ps[:, bass.ts(nch, 512)],
                                w_sb[wname][:, 2 * p2:2 * p2 + 2,
                                            bass.ts(mch, P)],
                                rhs_tile[:, 2 * p2:2 * p2 + 2,
                                         bass.ts(nch, 512)],
                                start=(p2 == 0), stop=(p2 == 1),
                                perf_mode=DR)

                with nc.named_scope("proj"):
                    for mch in range(KT):
                        ps_q = psP.tile([P, T], F32, tag="ps", name="ps")
                        proj_mm(ps_q, "wq", xlnT, mch)
                        nc.scalar.activation(
                            out=qu[:, mch, :], in_=ps_q[:], func=AF.Identity,
                            bias=u_sb[:, mch:mch + 1], scale=1.0 / SW)
                        nc.scalar.activation(
                            out=qv[:, mch, :], in_=ps_q[:], func=AF.Identity,
                            bias=v_sb[:, mch:mch + 1], scale=1.0 / SW)
                        ps_k = psP.tile([P, T], F32, tag="ps", name="ps")
                        proj_mm(ps_k, "wk", xlnT, mch)
                        nc.vector.tensor_scalar(
                            out=kT_sb[:, mch, :], in0=ps_k[:],
                            scalar1=1.0 / SW, scalar2=None, op0=AX.mult)
                        ps_p = psP.tile([P, T], F32, tag="ps", name="ps")
                        proj_mm(ps_p, "wp", post_sb, mch)
                        nc.vector.tensor_scalar(
                            out=pT_sb[:, mch, :], in0=ps_p[:],
                            scalar1=1.0 / SW, scalar2=None, op0=AX.mult)
                    # V projection into the packed (c, b) layout:
                    # out partition j covers token m = 256c + 2j + b.
                    for c in range(NC_XB):
                        for b in range(2):
                            ps_v = psV.tile([P, D], F32, tag="psv",
                                            name="psv")
                            tok0 = 256 * c + b
                            for p2 in range(2):
                                nc.tensor.matmul(
                                    ps_v[:],
                                    xlnT[:, 2 * p2:2 * p2 + 2,
                                         tok0:tok0 + 2 * P - 1:2],
                                    w_sb["wv"][:, 2 * p2:2 * p2 + 2, :],
                                    start=(p2 == 0), stop=(p2 == 1),
                                    perf_mode=DR)
                            nc.scalar.activation(
                                out=avw[:, c, b, :, 0:O],
                                in_=ps_v[:].rearrange("p (h o) -> p h o",
                                                      o=O),
                                func=AF.Copy, scale=1.0 / SW)

            # ====== attention: pipeline over head pairs ==========
            # phase p: BD+shift-write (pair p) | xbar read (pair p-1) |
            # AC+add+exp+AV+fin (pair p-2), interleaved.
            YROW = T + 1

            with (
                tc.tile_pool(name="ywr", bufs=2) as ywr_pool,
                tc.tile_pool(name="bd16", bufs=2) as bd16_pool,
                tc.tile_pool(name="et", bufs=2) as et_pool,
                tc.tile_pool(name="avsb", bufs=2) as avsb_pool,
                tc.tile_pool(name="ps_bd", bufs=2, space="PSUM") as ps_bd_pool,
                tc.tile_pool(name="ps_s", bufs=1, space="PSUM") as ps_s_pool,
                tc.tile_pool(name="ps_av", bufs=1, space="PSUM") as ps_av_pool,
            ):
                ydram_all = {}
                bd16_all = {}
                et_all = {}
                av_ps = {}

                def hbase(h):
                    return (h % 2) * O

                def emit_bd_nt(pair, nt, heads):
                    # BD_raw[n, m] for both heads of the pair; rows to DRAM
                    ywr = {}
                    for h in heads:
                        ywr[h] = ywr_pool.tile(
                            [P, YROW], FP8,
                            tag=f"ywr{h % 2}", name=f"ywr{h % 2}")
                        nc.gpsimd.memset(ywr[h][:, 0:1], 0.0)
                    for mch in range(NCH):
                        ps = {}
                        for h in heads:
                            base = hbase(h)
                            ps[h] = ps_bd_pool.tile([P, 512], F32, tag="ps",
                                                    name="ps")
                            nc.tensor.matmul(
                                ps[h][:],
                                qv[base:base + O, pair, bass.ts(nt, P)],
                                pT_sb[base:base + O, pair, bass.ts(mch, 512)],
                                start=True, stop=True)
                        for h in heads:
                            nc.vector.tensor_copy(
                                ywr[h][:, 1 + 512 * mch:1 + 512 * (mch + 1)],
                                ps[h][:])
                    for h in heads:
                        yv = ydram_all[pair][h][:].rearrange(
                            "(n c) -> n c", c=YROW)
                        nc.sync.dma_start(
                            yv[bass.ts(nt, P), :], ywr[h][:])

                def emit_xbar_read(pair, h, c):
                    # shifted+transposed read: 16-bit units pair fp8 bytes
                    # (m=256c+2j, m=256c+2j+1) -> partition j, byte b.
                    fb = ydram_all[pair][h][:].bitcast(BF16)
                    view = fb[T // 2:T // 2 + (T // 2) * T].rearrange(
                        "(n mp) -> n mp", mp=T // 2)
                    dst = bd16_all[pair][h][c]
                    nc.sync.dma_start(
                        dst[:], view[:, bass.ts(c, P)], transpose=True)

                def emit_acs_cb(pair, c, b, heads):
                    # one (c, b) slice for BOTH heads: BD enters PSUM via
                    # identity matmul, AC accumulates on top (start=False;
                    # heads pair-concurrent via row groups 0-1/2-3), then
                    # one [128, 1024] exp per head.
                    tok0 = 256 * c + b
                    ps = {}
                    bdp = {}
                    for h in heads:
                        ps[h] = ps_s_pool.tile([P, T], F32,
                                               tag=f"s{h % 2}",
                                               name=f"s{h % 2}")
                        bdp[h] = bd16_all[pair][h][c][:].bitcast(
                            FP8).rearrange("p (n b) -> p n b", b=2)
                    for h in heads:
                        for nch in range(NCH):
                            nc.tensor.matmul(
                                ps[h][:, bass.ts(nch, 512)], ident8[:],
                                bdp[h][:, bass.ts(nch, 512), b],
                                start=True, stop=False)
                    for nch in range(NCH):
                        for h in heads:
                            base = hbase(h)
                            nc.tensor.matmul(
                                ps[h][:, bass.ts(nch, 512)],
                                kT_sb[base:base + O, pair,
                                      tok0:tok0 + 2 * P - 1:2],
                                qu[base:base + O, pair, bass.ts(nch, 512)],
                                start=False, stop=True)
                    for h in heads:
                        nc.scalar.activation(
                            out=et_all[pair][h][:, c, b, :],
                            in_=ps[h][:], func=AF.Exp, scale=1.0 / 8.0)

                def emit_av(pair, h):
                    et = et_all[pair][h]
                    ps_nch = [
                        ps_av_pool.tile([O + 1, 512], F32,
                                        tag=f"av{nch}", name=f"av{nch}")
                        for nch in range(NCH)]
                    av_ps[(pair, h)] = ps_nch
                    for c in range(NC_XB):
                        for nch in range(NCH):
                            nc.tensor.matmul(
                                ps_nch[nch][0:O + 1, :],
                                avw[:, c, 0:2, h, 0:O + 1],
                                et[:, c, 0:2, bass.ts(nch, 512)],
                                start=(c == 0), stop=(c == NC_XB - 1),
                                perf_mode=DR)

                def emit_av_fin(pair, h):
                    base = hbase(h)
                    ps_nch = av_ps.pop((pair, h))
                    for nch in range(NCH):
                        av_sb = avsb_pool.tile([O + 1, 512], BF16,
                                               tag=f"avsb{h % 2}")
                        nc.scalar.copy(av_sb[:], ps_nch[nch][0:O + 1, :])
                        # broadcast den/16 into the just-freed psum tile
                        nc.tensor.matmul(
                            ps_nch[nch][0:O, :],
                            ones_bc[O:O + 1, :],
                            av_sb[O:O + 1, :],
                            start=True, stop=True)
                        rb = avsb_pool.tile([O, 512], F32, tag=f"rb{h % 2}")
                        nc.vector.reciprocal_approx_fast(
                            out=rb[:], in_=ps_nch[nch][0:O, :])
                        nc.gpsimd.tensor_tensor(
                            out=outT[base:base + O, pair, bass.ts(nch, 512)],
                            in0=av_sb[0:O, :], in1=rb[:], op=AX.mult)

                def s1_units(pair):
                    heads = (2 * pair, 2 * pair + 1)
                    return [(emit_bd_nt, (pair, nt, heads))
                            for nt in range(NT)]

                def s3_units(pair):
                    heads = (2 * pair, 2 * pair + 1)
                    units = []
                    for c in range(NC_XB):
                        for b in range(2):
                            units.append((emit_acs_cb, (pair, c, b, heads)))
                    for h in heads:
                        units.append((emit_av, (pair, h)))
                        units.append((emit_av_fin, (pair, h)))
                    return units

                for p in range(NPAIR + 2):
                    if p < NPAIR:
                        heads = (2 * p, 2 * p + 1)
                        ydram_all[p] = {
                            h: dram_pool.tile([T * YROW], FP8,
                                              tag=f"y{h % 2}", name=f"y{h % 2}")
                            for h in heads}
                        bd16_all[p] = {
                            h: [bd16_pool.tile([P, T], BF16,
                                               tag=f"bd{h % 2}c{c}",
                                               name=f"bd{h % 2}c{c}")
                                for c in range(NC_XB)]
                            for h in heads}
                        et_all[p] = {
                            h: et_pool.tile([P, NC_XB, 2, T], FP8,
                                            tag=f"et{h % 2}", name=f"et{h % 2}")
                            for h in heads}
                    if 1 <= p <= NPAIR:
                        for h in (2 * (p - 1), 2 * (p - 1) + 1):
                            for c in range(NC_XB):
                                emit_xbar_read(p - 1, h, c)
                    s1 = s1_units(p) if p < NPAIR else []
                    s3 = s3_units(p - 2) if 2 <= p else []
                    # proportional round-robin interleave
                    while s1 or s3:
                        if s1 and (not s3 or len(s1) * 12 >= len(s3) * 8):
                            fn, args = s1.pop(0)
                        else:
                            fn, args = s3.pop(0)
                        fn(*args)

            # ---- output projection + residual ----
            with (
                tc.tile_pool(name="fin", bufs=4) as fin_pool,
                tc.tile_pool(name="ps_y", bufs=4, space="PSUM") as ps_y_pool,
            ):
                with nc.named_scope("out"):
                    for nt in range(NT):
                        ps_y = ps_y_pool.tile([P, D], F32, tag="ps", name="ps")
                        for c2 in range(2):
                            nc.tensor.matmul(
                                ps_y[:],
                                outT[:, 2 * c2:2 * c2 + 2, bass.ts(nt, P)],
                                w_sb["wo"][:, 2 * c2:2 * c2 + 2, :],
                                start=(c2 == 0), stop=(c2 == 1),
                                perf_mode=DR)
                        fin = fin_pool.tile([P, D], BF16)
                        nc.vector.tensor_tensor(
                            out=fin[:], in0=ps_y[:], in1=xres_sb[:, nt, :],
                            op=AX.add)
                        nc.sync.dma_start(out[bass.ts(nt, P), :], fin[:])

    nc.compile()
    return nc


_NC = {}


def _get_nc(use_beta):
    if use_beta not in _NC:
        _NC[use_beta] = build_nc(use_beta)
    return _NC[use_beta]


def _run(inputs_dict, trace=False, trace_cores=None):
    bf = ml_dtypes.bfloat16
    f8 = ml_dtypes.float8_e4m3
    inputs = np.asarray(inputs_dict["inputs"], np.float32)
    pos = np.asarray(inputs_dict["pos"], np.float32)
    gamma = np.asarray(inputs_dict["gamma"], np.float32)
    beta = np.asarray(inputs_dict["beta"], np.float32)
    qk = np.asarray(inputs_dict["query_kernel"], np.float32)   # [H, D, O]
    kk = np.asarray(inputs_dict["key_kernel"], np.float32)
    vk = np.asarray(inputs_dict["value_kernel"], np.float32)
    pk = np.asarray(inputs_dict["pos_kernel"], np.float32)
    u = np.asarray(inputs_dict["pos_bias_u"], np.float32)      # [H, O]
    v = np.asarray(inputs_dict["pos_bias_v"], np.float32)
    prk = np.asarray(inputs_dict["projection_kernel"], np.float32)  # [H, O, D]
    pbias = np.asarray(inputs_dict["projection_bias"], np.float32)

    def wcat(w, rowscale=None):  # [H, D, O] -> [P, KT, (h o)], x16 fp8
        c = np.transpose(w, (1, 0, 2)).reshape(D, H * O) * SW
        if rowscale is not None:
            c = c * rowscale[:, None]
        return np.ascontiguousarray(
            c.reshape(KT, P, H * O).transpose(1, 0, 2)).astype(f8)

    wq_c = wcat(qk, gamma)
    wk_c = wcat(kk, gamma)
    wv_c = wcat(vk, gamma)
    wp_c = wcat(pk)
    wo_c = np.ascontiguousarray(
        (prk * SW).reshape(H * O, D).reshape(KT, P, D)
        .transpose(1, 0, 2)).astype(f8)
    u_c = np.ascontiguousarray(u.reshape(H * O).reshape(KT, P).T).astype(np.float32)
    v_c = np.ascontiguousarray(v.reshape(H * O).reshape(KT, P).T).astype(np.float32)
    beta_adj = np.where(gamma != 0, beta / np.where(gamma == 0, 1, gamma), 0.0)
    use_beta = bool(np.any(beta_adj != 0))
    # xln_nd is at TRUE scale (rstd absorbs the SR residual scaling)
    beta_b = np.broadcast_to(beta_adj[None, :], (P, D)).astype(bf).copy()

    in_maps = []
    for b in range(8):
        x_b = inputs[b] * SR
        m = {
            "x_res": np.ascontiguousarray(
                x_b.reshape(NT, P, D).transpose(1, 0, 2)).astype(bf),
            "post": np.ascontiguousarray(
                pos[b].T.reshape(KT, P, T).transpose(1, 0, 2)).astype(f8),
            "wq": wq_c, "wk": wk_c, "wv": wv_c, "wp": wp_c, "wo": wo_c,
            "u_in": u_c, "v_in": v_c,
        }
        if use_beta:
            m["beta_in"] = beta_b
        in_maps.append(m)

    nc = _get_nc(use_beta)
    res = run_bass_kernel_spmd(
        nc, in_maps, core_ids=list(range(8)), trace=trace,
        trace_cores=trace_cores)
    outs = np.stack([np.asarray(r["out"], np.float32) for r in res.results])
    outs = outs * (1.0 / SR) + pbias[None, None, :]
    return outs, res


def kernel(**inputs):
    outs, _ = _run(inputs)
    return outs


if __name__ == "__main__":
    nc = build_nc()
    print("built ok")


# revision 23
# speedup vs baseline: 1.1851x; 1.1851x over previous
"""Trainium2 Bass kernel for Conformer-style relative-position MHSA.

Sharding: data-parallel over batch — B=8 batch elements, one per NeuronCore.
Per core: LN -> QKVP projections -> rel-pos scores -> softmax -> AV -> output
projection -> residual. No collectives.

v2: scores are computed directly TRANSPOSED ([m on partitions, n free]) so
the AV contraction needs no PE transposes (baseline spent ~140us on 512
transpose instructions and HAM-oscillated around them):
  - AC^T[m,n] = k[.,m]^T q'[.,n] — same matmuls as AC with lhsT/rhs swapped.
  - BD needs the Transformer-XL shift, done via an fp8 DRAM round trip
    ([T, T+1] rows with a leading zero column). The shifted+transposed
    read-back uses the DMA X-bar transpose on a bf16 BITCAST of the fp8
    buffer: each 16-bit unit carries the fp8 pair (m=2j, m=2j+1), so the
    transposed SBUF tile holds m interleaved at byte granularity
    (m = 256*c + 2*j + b for xbar chunk c, partition j, byte b).
  - AC matmuls and the V projection select the matching stride-2 token
    subsets so everything downstream lives in the packed (c,j,b) layout.
  - K=64 score matmuls are plain fp8 (no DoubleRow — it only costs extra
    LDWEIGHTS here); the two heads of a pair sit at partition bases 0/64,
    so their matmuls row-group-tile and run concurrently on the PE.
Softmax denominator via an extra ones-column in the AV weights (row O of
the AV psum), divided out after. exp applies the 1/sqrt(64) scale.
fp8 fast path elsewhere as baseline: weights hostside at 16x, projections
DoubleRow over the d=512 contraction, residual path at 256x.
"""

import sys

for _p in ("/opt/trn_rl_repo", "/root/.axon_site/_ro/pypackages"):
    if _p not in sys.path:
        sys.path.insert(0, _p)

import numpy as np
import ml_dtypes

import concourse.bass as bass
import concourse.mybir as mybir
import concourse.tile as tile
from concourse import bacc
from concourse.bass_utils import run_bass_kernel_spmd
from concourse.masks import make_identity

F32 = mybir.dt.float32
BF16 = mybir.dt.bfloat16
FP8 = mybir.dt.float8e4
AX = mybir.AluOpType
AF = mybir.ActivationFunctionType
DR = mybir.MatmulPerfMode.DoubleRow

P = 128
T = 1024
D = 512
H = 8
O = 64
KT = D // P      # 4 k-tiles over model dim
NT = T // P      # 8 tiles over sequence
NCH = T // 512   # 2 free-dim chunks of 512
NC_XB = 4        # xbar chunks of 256 m-values (128 fp8 pairs)
AVP = 80         # avw per-head pitch (ones col at 64; stride % 16 == 0)
NPAIR = H // 2
LN_EPS = 1e-3
SW = 16.0        # weight fp8 scale
SR = 256.0       # residual-path scale (SW*SW)


def build_nc(use_beta=True):
    nc = bacc.Bacc("TRN2", target_bir_lowering=False)

    x_res = nc.dram_tensor("x_res", [P, NT, D], BF16, kind="ExternalInput")
    post = nc.dram_tensor("post", [P, KT, T], FP8, kind="ExternalInput")
    wq = nc.dram_tensor("wq", [P, KT, D], FP8, kind="ExternalInput")
    wk = nc.dram_tensor("wk", [P, KT, D], FP8, kind="ExternalInput")
    wv = nc.dram_tensor("wv", [P, KT, D], FP8, kind="ExternalInput")
    wp = nc.dram_tensor("wp", [P, KT, D], FP8, kind="ExternalInput")
    wo = nc.dram_tensor("wo", [P, KT, D], FP8, kind="ExternalInput")
    u_in = nc.dram_tensor("u_in", [P, KT], F32, kind="ExternalInput")
    v_in = nc.dram_tensor("v_in", [P, KT], F32, kind="ExternalInput")
    if use_beta:
        beta_in = nc.dram_tensor("beta_in", [P, D], BF16,
                                 kind="ExternalInput")
    out = nc.dram_tensor("out", [T, D], BF16, kind="ExternalOutput")

    with tile.TileContext(nc) as tc:
        with (
            tc.tile_pool(name="consts", bufs=1) as consts,
            tc.tile_pool(name="acts", bufs=1) as acts,
            tc.tile_pool(name="dram", bufs=2, space="DRAM") as dram_pool,
        ):
            xres_sb = acts.tile([P, NT, D], BF16)
            nc.sync.dma_start(xres_sb[:], x_res[:])
            if use_beta:
                beta_sb = consts.tile([P, D], BF16, tag="beta")
                nc.sync.dma_start(beta_sb[:], beta_in[:])
            ones_bc = consts.tile([P, O], BF16, tag="ones_bc")
            nc.vector.memset(ones_bc[:], 1.0 / SW)
            eps_sb = consts.tile([P, 1], F32, tag="eps")
            nc.vector.memset(eps_sb[:], LN_EPS * SR * SR)
            ident = consts.tile([P, P], BF16)
            make_identity(nc, ident)
            ident8 = consts.tile([P, P], FP8, tag="ident8")
            make_identity(nc, ident8)

            # projections land transposed: [feat-part, chunk, token]
            qu = acts.tile([P, KT, T], FP8)
            qv = acts.tile([P, KT, T], FP8)
            kT_sb = acts.tile([P, KT, T], FP8)
            pT_sb = acts.tile([P, KT, T], FP8)
            outT = acts.tile([P, KT, T], FP8)
            # AV weights, packed: [j, c, b, h, o] with m = 256c + 2j + b
            avw = acts.tile([P, NC_XB, 2, H, AVP], FP8)
            nc.vector.memset(avw[:], 1.0)

            with (
                tc.tile_pool(name="early", bufs=1) as early,
                tc.tile_pool(name="psP", bufs=2, space="PSUM") as psP,
                tc.tile_pool(name="psV", bufs=2, space="PSUM") as psV,
                tc.tile_pool(name="psB", bufs=2, space="PSUM") as psB,
            ):
                xlnT = early.tile([P, KT, T], FP8)
                xln_nd = early.tile([P, NT, D], BF16)
                with tc.tile_pool(name="ln_tmp", bufs=4) as ln_tmp:
                    with nc.named_scope("ln"):
                        for nt in range(NT):
                            st6 = ln_tmp.tile([P, 6], F32, tag="st6")
                            nc.vector.bn_stats(out=st6[:], in_=xres_sb[:, nt, :])
                            mv = ln_tmp.tile([P, 2], F32, tag="mv")
                            nc.vector.bn_aggr(out=mv[:], in_=st6[:])
                            sd = ln_tmp.tile([P, 1], F32, tag="sd")
                            nc.scalar.activation(out=sd[:], in_=mv[:, 1:2],
                                                 func=AF.Sqrt, bias=eps_sb[:])
                            rstd = ln_tmp.tile([P, 1], F32, tag="rstd")
                            nc.vector.reciprocal(rstd[:], sd[:])
                            nc.vector.tensor_scalar(
                                out=xln_nd[:, nt, :], in0=xres_sb[:, nt, :],
                                scalar1=mv[:, 0:1], scalar2=rstd[:],
                                op0=AX.subtract, op1=AX.mult)
                            if use_beta:
                                nc.vector.tensor_add(
                                    xln_nd[:, nt, :], xln_nd[:, nt, :],
                                    beta_sb[:])
                        for kt in range(KT):
                            ps_x = psB.tile([P, T], BF16, tag="tx")
                            for nt in range(NT):
                                nc.tensor.transpose(
                                    ps_x[:, bass.ts(nt, P)],
                                    xln_nd[:, nt, bass.ts(kt, P)],
                                    ident[:])
                            nc.scalar.copy(xlnT[:, kt, :], ps_x[:])

                post_sb = early.tile([P, KT, T], FP8)
                nc.sync.dma_start(post_sb[:], post[:])
                w_sb = {}
                for name, t in (("wq", wq), ("wk", wk), ("wv", wv), ("wp", wp),
                                ("wo", wo)):
                    w_sb[name] = consts.tile([P, KT, D], FP8, tag=f"w_{name}",
                                             name=f"w_{name}")
                    nc.sync.dma_start(w_sb[name][:], t[:])
                u_sb = consts.tile([P, KT], F32, tag="u")
                nc.sync.dma_start(u_sb[:], u_in[:])
                v_sb = consts.tile([P, KT], F32, tag="v")
                nc.sync.dma_start(v_sb[:], v_in[:])

                # ---- projections (DoubleRow over kt pairs) ----
                def proj_mm(ps, wname, rhs_tile, mch):
                    for nch in range(NCH):
                        for p2 in range(2):
                            nc.tensor.matmul(
                                ps[:, bass.ts(nch, 512)],
                                w_sb[wname][:, 2 * p2:2 * p2 + 2,
                                            bass.ts(mch, P)],
                                rhs_tile[:, 2 * p2:2 * p2 + 2,
                                         bass.ts(nch, 512)],
                                start=(p2 == 0), stop=(p2 == 1),
                                perf_mode=DR)

                with nc.named_scope("proj"):
                    for mch in range(KT):
                        ps_q = psP.tile([P, T], F32, tag="ps", name="ps")
                        proj_mm(ps_q, "wq", xlnT, mch)
                        nc.scalar.activation(
                            out=qu[:, mch, :], in_=ps_q[:], func=AF.Identity,
                            bias=u_sb[:, mch:mch + 1], scale=1.0 / SW)
                        nc.scalar.activation(
                            out=qv[:, mch, :], in_=ps_q[:], func=AF.Identity,
                            bias=v_sb[:, mch:mch + 1], scale=1.0 / SW)
                        ps_k = psP.tile([P, T], F32, tag="ps", name="ps")
                        proj_mm(ps_k, "wk", xlnT, mch)
                        nc.vector.tensor_scalar(
                            out=kT_sb[:, mch, :], in0=ps_k[:],
                            scalar1=1.0 / SW, scalar2=None, op0=AX.mult)
                        ps_p = psP.tile([P, T], F32, tag="ps", name="ps")
                        proj_mm(ps_p, "wp", post_sb, mch)
                        nc.vector.tensor_scalar(
                            out=pT_sb[:, mch, :], in0=ps_p[:],
                            scalar1=1.0 / SW, scalar2=None, op0=AX.mult)
                    # V projection into the packed (c, b) layout:
                    # out partition j covers token m = 256c + 2j + b.
                    for c in range(NC_XB):
                        for b in range(2):
                            ps_v = psV.tile([P, D], F32, tag="psv",
                                            name="psv")
                            tok0 = 256 * c + b
                            for p2 in range(2):
                                nc.tensor.matmul(
                                    ps_v[:],
                                    xlnT[:, 2 * p2:2 * p2 + 2,
                                         tok0:tok0 + 2 * P - 1:2],
                                    w_sb["wv"][:, 2 * p2:2 * p2 + 2, :],
                                    start=(p2 == 0), stop=(p2 == 1),
                                    perf_mode=DR)
                            nc.scalar.activation(
                                out=avw[:, c, b, :, 0:O],
                                in_=ps_v[:].rearrange("p (h o) -> p h o",
                                                      o=O),
                                func=AF.Copy, scale=1.0 / SW)

            # ====== attention: pipeline over head pairs ==========
            # phase p: BD+shift-write (pair p) | xbar read (pair p-1) |
            # AC+add+exp+AV+fin (pair p-2), interleaved.
            YROW = T + 1

            with (
                tc.tile_pool(name="ywr", bufs=2) as ywr_pool,
                tc.tile_pool(name="bd16", bufs=2) as bd16_pool,
                tc.tile_pool(name="et", bufs=2) as et_pool,
                tc.tile_pool(name="avsb", bufs=2) as avsb_pool,
                tc.tile_pool(name="ps_bd", bufs=2, space="PSUM") as ps_bd_pool,
                tc.tile_pool(name="ps_s", bufs=1, space="PSUM") as ps_s_pool,
                tc.tile_pool(name="ps_av", bufs=1, space="PSUM") as ps_av_pool,
            ):
                ydram_all = {}
                bd16_all = {}
                et_all = {}
                av_ps = {}

                def hbase(h):
                    return (h % 2) * O

                def emit_bd_nt(pair, nt, heads, scalar_help):
                    # BD_raw[n, m] for both heads of the pair; rows to DRAM
                    ywr = {}
                    for h in heads:
                        ywr[h] = ywr_pool.tile(
                            [P, YROW], FP8,
                            tag=f"ywr{h % 2}", name=f"ywr{h % 2}")
                        nc.gpsimd.memset(ywr[h][:, 0:1], 0.0)
                    for mch in range(NCH):
                        ps = {}
                        for h in heads:
                            base = hbase(h)
                            ps[h] = ps_bd_pool.tile([P, 512], F32, tag="ps",
                                                    name="ps")
                            nc.tensor.matmul(
                                ps[h][:],
                                qv[base:base + O, pair, bass.ts(nt, P)],
                                pT_sb[base:base + O, pair, bass.ts(mch, 512)],
                                start=True, stop=True)
                        for h in heads:
                            dst = ywr[h][:, 1 + 512 * mch:1 + 512 * (mch + 1)]
                            if scalar_help and (h + mch) % 2 == 0:
                                nc.scalar.copy(dst, ps[h][:])
                            else:
                                nc.vector.tensor_copy(dst, ps[h][:])
                    for h in heads:
                        yv = ydram_all[pair][h][:].rearrange(
                            "(n c) -> n c", c=YROW)
                        nc.sync.dma_start(
                            yv[bass.ts(nt, P), :], ywr[h][:])

                def emit_xbar_read(pair, h, c):
                    # shifted+transposed read: 16-bit units pair fp8 bytes
                    # (m=256c+2j, m=256c+2j+1) -> partition j, byte b.
                    fb = ydram_all[pair][h][:].bitcast(BF16)
                    view = fb[T // 2:T // 2 + (T // 2) * T].rearrange(
                        "(n mp) -> n mp", mp=T // 2)
                    dst = bd16_all[pair][h][c]
                    nc.sync.dma_start(
                        dst[:], view[:, bass.ts(c, P)], transpose=True)

                def emit_acs_cb(pair, c, b, heads):
                    # one (c, b) slice for BOTH heads: BD enters PSUM via
                    # identity matmul, AC accumulates on top (start=False;
                    # heads pair-concurrent via row groups 0-1/2-3), then
                    # one [128, 1024] exp per head.
                    tok0 = 256 * c + b
                    ps = {}
                    bdp = {}
                    for h in heads:
                        ps[h] = ps_s_pool.tile([P, T], F32,
                                               tag=f"s{h % 2}",
                                               name=f"s{h % 2}")
                        bdp[h] = bd16_all[pair][h][c][:].bitcast(
                            FP8).rearrange("p (n b) -> p n b", b=2)
                    for h in heads:
                        for nch in range(NCH):
                            nc.tensor.matmul(
                                ps[h][:, bass.ts(nch, 512)], ident8[:],
                                bdp[h][:, bass.ts(nch, 512), b],
                                start=True, stop=False)
                    for nch in range(NCH):
                        for h in heads:
                            base = hbase(h)
                            nc.tensor.matmul(
                                ps[h][:, bass.ts(nch, 512)],
                                kT_sb[base:base + O, pair,
                                      tok0:tok0 + 2 * P - 1:2],
                                qu[base:base + O, pair, bass.ts(nch, 512)],
                                start=False, stop=True)
                    for h in heads:
                        nc.scalar.activation(
                            out=et_all[pair][h][:, c, b, :],
                            in_=ps[h][:], func=AF.Exp, scale=1.0 / 8.0)

                def emit_av(pair, h):
                    et = et_all[pair][h]
                    ps_nch = [
                        ps_av_pool.tile([O + 1, 512], F32,
                                        tag=f"av{nch}", name=f"av{nch}")
                        for nch in range(NCH)]
                    av_ps[(pair, h)] = ps_nch
                    for c in range(NC_XB):
                        for nch in range(NCH):
                            nc.tensor.matmul(
                                ps_nch[nch][0:O + 1, :],
                                avw[:, c, 0:2, h, 0:O + 1],
                                et[:, c, 0:2, bass.ts(nch, 512)],
                                start=(c == 0), stop=(c == NC_XB - 1),
                                perf_mode=DR)

                def emit_av_fin(pair, h):
                    base = hbase(h)
                    ps_nch = av_ps.pop((pair, h))
                    for nch in range(NCH):
                        av_sb = avsb_pool.tile([O + 1, 512], BF16,
                                               tag=f"avsb{h % 2}")
                        nc.scalar.copy(av_sb[:], ps_nch[nch][0:O + 1, :])
                        # broadcast den/16 into the just-freed psum tile
                        nc.tensor.matmul(
                            ps_nch[nch][0:O, :],
                            ones_bc[O:O + 1, :],
                            av_sb[O:O + 1, :],
                            start=True, stop=True)
                        rb = avsb_pool.tile([O, 512], F32, tag=f"rb{h % 2}")
                        nc.vector.reciprocal_approx_fast(
                            out=rb[:], in_=ps_nch[nch][0:O, :])
                        nc.vector.tensor_tensor(
                            out=outT[base:base + O, pair, bass.ts(nch, 512)],
                            in0=av_sb[0:O, :], in1=rb[:], op=AX.mult)

                def s1_units(pair):
                    heads = (2 * pair, 2 * pair + 1)
                    scalar_help = pair < 2  # before exp work ramps up
                    return [(emit_bd_nt, (pair, nt, heads, scalar_help))
                            for nt in range(NT)]

                def s3_units(pair):
                    heads = (2 * pair, 2 * pair + 1)
                    units = []
                    for c in range(NC_XB):
                        for b in range(2):
                            units.append((emit_acs_cb, (pair, c, b, heads)))
                    for h in heads:
                        units.append((emit_av, (pair, h)))
                        units.append((emit_av_fin, (pair, h)))
                    return units

                for p in range(NPAIR + 2):
                    if p < NPAIR:
                        heads = (2 * p, 2 * p + 1)
                        ydram_all[p] = {
                            h: dram_pool.tile([T * YROW], FP8,
                                              tag=f"y{h % 2}", name=f"y{h % 2}")
                            for h in heads}
                        bd16_all[p] = {
                            h: [bd16_pool.tile([P, T], BF16,
                                               tag=f"bd{h % 2}c{c}",
                                               name=f"bd{h % 2}c{c}")
                                for c in range(NC_XB)]
                            for h in heads}
                        et_all[p] = {
                            h: et_pool.tile([P, NC_XB, 2, T], FP8,
                                            tag=f"et{h % 2}", name=f"et{h % 2}")
                            for h in heads}
                    if 1 <= p <= NPAIR:
                        for h in (2 * (p - 1), 2 * (p - 1) + 1):
                            for c in range(NC_XB):
                                emit_xbar_read(p - 1, h, c)
                    s1 = s1_units(p) if p < NPAIR else []
                    s3 = s3_units(p - 2) if 2 <= p else []
                    # proportional round-robin interleave
                    while s1 or s3:
                        if s1 and (not s3 or len(s1) * 12 >= len(s3) * 8):
                            fn, args = s1.pop(0)
                        else:
                            fn, args = s3.pop(0)
                        fn(*args)

            # ---- output projection + residual ----
            with (
                tc.tile_pool(name="fin", bufs=4) as fin_pool,
                tc.tile_pool(name="ps_y", bufs=4, space="PSUM") as ps_y_pool,
            ):
                with nc.named_scope("out"):
                    for nt in range(NT):
                        ps_y = ps_y_pool.tile([P, D], F32, tag="ps", name="ps")
                        for c2 in range(2):
                            nc.tensor.matmul(
                                ps_y[:],
                                outT[:, 2 * c2:2 * c2 + 2, bass.ts(nt, P)],
                                w_sb["wo"][:, 2 * c2:2 * c2 + 2, :],
                                start=(c2 == 0), stop=(c2 == 1),
                                perf_mode=DR)
                        fin = fin_pool.tile([P, D], BF16)
                        nc.vector.tensor_tensor(
                            out=fin[:], in0=ps_y[:], in1=xres_sb[:, nt, :],
                            op=AX.add)
                        nc.sync.dma_start(out[bass.ts(nt, P), :], fin[:])

    nc.compile()
    return nc


_NC = {}


def _get_nc(use_beta):
    if use_beta not in _NC:
        _NC[use_beta] = build_nc(use_beta)
    return _NC[use_beta]


def _run(inputs_dict, trace=False, trace_cores=None):
    bf = ml_dtypes.bfloat16
    f8 = ml_dtypes.float8_e4m3
    inputs = np.asarray(inputs_dict["inputs"], np.float32)
    pos = np.asarray(inputs_dict["pos"], np.float32)
    gamma = np.asarray(inputs_dict["gamma"], np.float32)
    beta = np.asarray(inputs_dict["beta"], np.float32)
    qk = np.asarray(inputs_dict["query_kernel"], np.float32)   # [H, D, O]
    kk = np.asarray(inputs_dict["key_kernel"], np.float32)
    vk = np.asarray(inputs_dict["value_kernel"], np.float32)
    pk = np.asarray(inputs_dict["pos_kernel"], np.float32)
    u = np.asarray(inputs_dict["pos_bias_u"], np.float32)      # [H, O]
    v = np.asarray(inputs_dict["pos_bias_v"], np.float32)
    prk = np.asarray(inputs_dict["projection_kernel"], np.float32)  # [H, O, D]
    pbias = np.asarray(inputs_dict["projection_bias"], np.float32)

    def wcat(w, rowscale=None):  # [H, D, O] -> [P, KT, (h o)], x16 fp8
        c = np.transpose(w, (1, 0, 2)).reshape(D, H * O) * SW
        if rowscale is not None:
            c = c * rowscale[:, None]
        return np.ascontiguousarray(
            c.reshape(KT, P, H * O).transpose(1, 0, 2)).astype(f8)

    wq_c = wcat(qk, gamma)
    wk_c = wcat(kk, gamma)
    wv_c = wcat(vk, gamma)
    wp_c = wcat(pk)
    wo_c = np.ascontiguousarray(
        (prk * SW).reshape(H * O, D).reshape(KT, P, D)
        .transpose(1, 0, 2)).astype(f8)
    u_c = np.ascontiguousarray(u.reshape(H * O).reshape(KT, P).T).astype(np.float32)
    v_c = np.ascontiguousarray(v.reshape(H * O).reshape(KT, P).T).astype(np.float32)
    beta_adj = np.where(gamma != 0, beta / np.where(gamma == 0, 1, gamma), 0.0)
    use_beta = bool(np.any(beta_adj != 0))
    # xln_nd is at TRUE scale (rstd absorbs the SR residual scaling)
    beta_b = np.broadcast_to(beta_adj[None, :], (P, D)).astype(bf).copy()

    in_maps = []
    for b in range(8):
        x_b = inputs[b] * SR
        m = {
            "x_res": np.ascontiguousarray(
                x_b.reshape(NT, P, D).transpose(1, 0, 2)).astype(bf),
            "post": np.ascontiguousarray(
                pos[b].T.reshape(KT, P, T).transpose(1, 0, 2)).astype(f8),
            "wq": wq_c, "wk": wk_c, "wv": wv_c, "wp": wp_c, "wo": wo_c,
            "u_in": u_c, "v_in": v_c,
        }
        if use_beta:
            m["beta_in"] = beta_b
        in_maps.append(m)

    nc = _get_nc(use_beta)
    res = run_bass_kernel_spmd(
        nc, in_maps, core_ids=list(range(8)), trace=trace,
        trace_cores=trace_cores)
    outs = np.stack([np.asarray(r["out"], np.float32) for r in res.results])
    outs = outs * (1.0 / SR) + pbias[None, None, :]
    return outs, res


def kernel(**inputs):
    outs, _ = _run(inputs)
    return outs


if __name__ == "__main__":
    nc = build_nc()
    print("built ok")
